# revision 1
# baseline (speedup 1.0000x reference)
"""Trainium2 Bass kernel v2 for nn_Encoder_88656714924838.

6-layer encoder, d_model=64, 4 heads x dk=16, d_ff=512, B=256, L=128.
Data parallel over 8 cores (32 batches/core). Device kernel does all layers.

v2 layout ideas vs baseline:
- 2-batch pair transposes ([128,128] PE transposes, full partition width).
- Head-padded K^T/Q^T at 32-aligned partition strips; per-head score matmuls
  via PE tile_position (row strip 32h) - no padded-Q 4x waste.
- Weights duplicated on both partition halves so odd-pair-half operands can
  use tile_position=(64,0).
- Group-batched PSUM evacuation (4-batch banks), broadcast LN apply,
  Pool engine takes transpose-copies.
"""

import sys

for _p in ("/opt/trn_rl_repo",):
    if _p not in sys.path:
        sys.path.insert(0, _p)

import numpy as np

D_MODEL = 64
N_HEADS = 4
D_K = 16
D_FF = 512
N_LAYERS = 6
B, L = 256, 128
N_CORES = 8
B_LOC = B // N_CORES
SCALE = 1.0 / np.sqrt(np.float32(D_K))

G = 16  # batches per group
NPAIR = G // 2
NQUAD = G // 4  # quads of 4 batches per group


def _positional_encoding(length=L, d_model=D_MODEL):
    pos = np.arange(length, dtype=np.float32)[:, None]
    div = np.exp(
        np.arange(0, d_model, 2, dtype=np.float32) * (-np.log(10000.0) / d_model)
    )
    pe = np.zeros((length, d_model), dtype=np.float32)
    pe[:, 0::2] = np.sin(pos * div)
    pe[:, 1::2] = np.cos(pos * div)
    return pe


def _quad_j0(qi):
    """First local batch index of quad qi; batches are j0, j0+2, j0+4, j0+6."""
    par = qi % 2
    pairbase = (qi // 2) * 4
    return 2 * pairbase + par


def _split_multi_waits(nc):
    """walrus accepts only ONE sync-wait per instruction; hoist extras onto
    same-engine NoOps just before the carrying instruction."""
    import concourse.mybir as mybir

    k = 0
    for fn in nc.m.functions:
        for blk in fn.blocks:
            new = []
            changed = False
            for inst in blk.instructions:
                si = inst.sync_info
                waits = list(si.on_wait) if (si and si.on_wait) else []
                if len(waits) > 1:
                    changed = True
                    for w in waits[:-1]:
                        k += 1
                        nop = mybir.InstNoOp(name=f"ws-{k}", ins=[], outs=[])
                        nop.engine = inst.engine
                        nop.sync_info = mybir.SyncInfo(on_wait=[w], on_update=[])
                        nc.register_instruction(nop)
                        new.append(nop)
                    si.on_wait = waits[-1:]
                new.append(inst)
            if changed:
                blk.instructions = new


def _pad_heads_blocks(w):
    """[n, 64, 64] -> [n, 64, 4, 64]: block h keeps only head h's 16 cols."""
    n = w.shape[0]
    out = np.zeros((n, D_MODEL, N_HEADS, D_MODEL), dtype=np.float32)
    for h in range(N_HEADS):
        sl = slice(D_K * h, D_K * (h + 1))
        out[:, :, h, sl] = w[:, :, sl]
    return out


def _host_prep(inputs):
    import ml_dtypes

    enc = np.asarray(inputs["enc_inputs"])
    deg = np.asarray(inputs["degree_s"])
    MD = np.asarray(inputs["MD"])
    src_emb = np.asarray(inputs["src_emb"], dtype=np.float32)
    deg_emb = np.asarray(inputs["deg_emb"], dtype=np.float32)
    md_emb = np.asarray(inputs["md_emb"], dtype=np.float32)

    x0 = (src_emb[enc] + deg_emb[deg] + _positional_encoding()[None]).astype(
        np.float32
    )

    # scores^T layout [b, j, h, i]; fold key pad-mask; exponentiate.
    bias_t = np.ascontiguousarray(md_emb[MD].transpose(0, 2, 3, 1))
    mask = np.where(enc == 0, np.float32(-1e9), np.float32(0.0))
    with np.errstate(under="ignore"):
        ebt = np.exp(bias_t + mask[:, :, None, None], dtype=np.float32)
    ebt = ebt.astype(ml_dtypes.bfloat16)

    bf = ml_dtypes.bfloat16
    wq = _pad_heads_blocks(np.asarray(inputs["Wq"], dtype=np.float32) * SCALE).astype(bf)
    wk = np.asarray(inputs["Wk"], dtype=np.float32).astype(bf)
    wv = np.asarray(inputs["Wv"], dtype=np.float32).astype(bf)
    wo = np.asarray(inputs["Wo"], dtype=np.float32)
    w1 = np.asarray(inputs["W1"], dtype=np.float32).astype(bf)
    # W2 [n, 512, 64] -> [128, n, 4, 64] (c-chunk on partition)
    w2 = np.ascontiguousarray(
        np.asarray(inputs["W2"], dtype=np.float32)
        .reshape(N_LAYERS, 4, 128, D_MODEL)
        .transpose(2, 0, 1, 3)
    ).astype(bf)
    return x0, ebt, wq, wk, wv, wo, w1, w2



def build_nc(n_layers=N_LAYERS, b_loc=B_LOC):
    """All PE operands at partition base 0 (nonzero tile_position crashes
    this runtime - verified by bisection). Scores use the padded-Q trick."""
    import concourse.bass as bass
    import concourse.mybir as mybir
    import concourse.tile as tile
    from concourse.masks import make_identity

    f32 = mybir.dt.float32
    bf16 = mybir.dt.bfloat16
    AF = mybir.ActivationFunctionType

    nc = bass.Bass("TRN2", target_bir_lowering=False, debug=False)

    x0_d = nc.dram_tensor("x0", [b_loc, L, D_MODEL], f32, kind="ExternalInput")
    ebt_d = nc.dram_tensor("ebt", [b_loc, L, N_HEADS, L], bf16, kind="ExternalInput")
    wq_d = nc.dram_tensor("wq", [n_layers, D_MODEL, N_HEADS, D_MODEL], bf16,
                          kind="ExternalInput")
    wk_d = nc.dram_tensor("wk", [n_layers, D_MODEL, D_MODEL], bf16,
                          kind="ExternalInput")
    wv_d = nc.dram_tensor("wv", [n_layers, D_MODEL, D_MODEL], bf16,
                          kind="ExternalInput")
    wo_d = nc.dram_tensor("wo", [n_layers, D_MODEL, D_MODEL], f32,
                          kind="ExternalInput")
    w1_d = nc.dram_tensor("w1", [n_layers, D_MODEL, D_FF], bf16,
                          kind="ExternalInput")
    w2_d = nc.dram_tensor("w2", [128, n_layers, 4, D_MODEL], bf16,
                          kind="ExternalInput")
    out_d = nc.dram_tensor("out", [b_loc, L, D_MODEL], f32, kind="ExternalOutput")

    n_groups = b_loc // G

    with tile.TileContext(nc) as tc:
        with (
            tc.tile_pool(name="consts", bufs=1) as consts,
            tc.tile_pool(name="state", bufs=1) as state,
            tc.tile_pool(name="work", bufs=2) as work,
            tc.tile_pool(name="pa", bufs=2, space="PSUM") as pa,
            tc.tile_pool(name="pst", bufs=2, space="PSUM") as pst,
            tc.tile_pool(name="pc", bufs=2, space="PSUM") as pc,
        ):
            ident = consts.tile([128, 128], f32)
            make_identity(nc, ident[:])
            identb = consts.tile([128, 128], bf16)
            nc.vector.tensor_copy(out=identb[:], in_=ident[:])
            eps_t = consts.tile([128, 1], f32)
            nc.vector.memset(eps_t[:], 1e-5)

            wq_sb = consts.tile([D_MODEL, n_layers, N_HEADS, D_MODEL], bf16)
            nc.sync.dma_start(out=wq_sb[:], in_=wq_d.rearrange("n k h m -> k n h m"))
            wk_sb = consts.tile([D_MODEL, n_layers, D_MODEL], bf16)
            nc.sync.dma_start(out=wk_sb[:], in_=wk_d.rearrange("n k m -> k n m"))
            wv_sb = consts.tile([D_MODEL, n_layers, D_MODEL], bf16)
            nc.sync.dma_start(out=wv_sb[:], in_=wv_d.rearrange("n k m -> k n m"))
            wo_sb = consts.tile([D_MODEL, n_layers, D_MODEL], f32)
            nc.sync.dma_start(out=wo_sb[:], in_=wo_d.rearrange("n k m -> k n m"))
            w1_sb = consts.tile([D_MODEL, n_layers, D_FF], bf16)
            nc.sync.dma_start(out=w1_sb[:], in_=w1_d.rearrange("n k m -> k n m"))
            w2_sb = consts.tile([128, n_layers, 4, D_MODEL], bf16)
            nc.sync.dma_start(out=w2_sb[:], in_=w2_d.ap())

            xs, vps = [], []
            for gi in range(n_groups):
                xg = state.tile([128, G, D_MODEL], f32, tag=f"x{gi}")
                nc.sync.dma_start(
                    out=xg[:],
                    in_=x0_d[gi * G : (gi + 1) * G].rearrange("b l d -> l b d"),
                )
                xs.append(xg)
                vg = state.tile([128, G, N_HEADS, D_K + 1], bf16, tag=f"vp{gi}")
                nc.vector.memset(vg[:, :, :, D_K : D_K + 1], 1.0)
                vps.append(vg)
            eb_sb = state.tile([128, b_loc, N_HEADS, L], bf16)
            for b in range(b_loc):
                nc.sync.dma_start(out=eb_sb[:, b, :, :], in_=ebt_d[b])

            for layer in range(n_layers):
                for g in range(n_groups):
                    x_g = xs[g]
                    vp = vps[g]
                    gb = g * G

                    # ---- A: per-batch transposes -> xt [64, G, 128]
                    xt = work.tile([64, G, 128], bf16, tag="xt")
                    for qj in range(G // 4):
                        tp = pa.tile([64, 4, 128], f32, tag="pab")
                        for s in range(4):
                            nc.tensor.transpose(
                                out=tp[:, s, :], in_=x_g[:, 4 * qj + s, :],
                                identity=ident[:],
                            )
                        nc.vector.tensor_copy(
                            out=xt[:, 4 * qj : 4 * qj + 4, :], in_=tp[:]
                        )

                    # ---- B: K (1 mm/chunk), Q padded (4 mm/chunk), V (4 mm/chunk)
                    kt = work.tile([64, NQUAD, 4, 128], bf16, tag="kt")
                    qt = work.tile([64, N_HEADS, NQUAD, 4, 128], bf16, tag="qt", bufs=1)
                    for qi in range(NQUAD):
                        j0 = 4 * qi
                        k_ps = pa.tile([64, 512], f32, tag="pa")
                        nc.tensor.matmul(
                            out=k_ps[:], lhsT=wk_sb[:, layer, :],
                            rhs=xt[:, j0 : j0 + 4, :], start=True, stop=True,
                        )
                        nc.scalar.copy(
                            out=kt[:, qi, :, :],
                            in_=k_ps[:].rearrange("p (s t) -> p s t", s=4),
                        )
                        for h in range(N_HEADS):
                            q_ps = pa.tile([64, 512], f32, tag="pa")
                            nc.tensor.matmul(
                                out=q_ps[:], lhsT=wq_sb[:, layer, h, :],
                                rhs=xt[:, j0 : j0 + 4, :], start=True, stop=True,
                            )
                            qeng = (nc.vector.tensor_copy, nc.scalar.copy,
                                    nc.vector.tensor_copy, nc.scalar.copy)[h]
                            qeng(
                                out=qt[:, h, qi, :, :],
                                in_=q_ps[:].rearrange("p (s t) -> p s t", s=4),
                            )
                        v_ps = pa.tile([128, 4, D_MODEL], f32, tag="pa")
                        for s in range(4):
                            nc.tensor.matmul(
                                out=v_ps[:, s, :], lhsT=xt[:, j0 + s, :],
                                rhs=wv_sb[:, layer, :], start=True, stop=True,
                            )
                        nc.scalar.copy(
                            out=vp[:, j0 : j0 + 4, :, 0:D_K],
                            in_=v_ps[:].rearrange("p s (h e) -> p s h e", h=N_HEADS),
                        )

                    # ---- C: scores -> exp -> * exp(bias)
                    at_q = []
                    for qi in range(NQUAD):
                        j0 = 4 * qi
                        ex = work.tile([128, 4, N_HEADS, 128], bf16, tag="ex", bufs=4)
                        for s in range(4):
                            st_ps = pst.tile([128, N_HEADS, 128], f32, tag="pst")
                            nc.tensor.matmul(
                                out=st_ps[:], lhsT=kt[:, qi, s, :],
                                rhs=qt[:, :, qi, s, :], start=True, stop=True,
                            )
                            nc.scalar.activation(
                                out=ex[:, s, :, :], in_=st_ps[:], func=AF.Exp
                            )
                        at = work.tile([128, 4, N_HEADS, 128], bf16, tag="at", bufs=4)
                        for s in range(4):
                            nc.gpsimd.tensor_mul(
                                out=at[:, s], in0=ex[:, s],
                                in1=eb_sb[:, gb + j0 + s, :, :],
                            )
                        at_q.append(at)

                    # ---- D: ctx = at @ [V|1]; normalize -> ctx_g
                    ctx_g = work.tile([128, G, D_MODEL], f32, tag="ctx")
                    for qi in range(NQUAD):
                        j0 = 4 * qi
                        ctx_ps = pc.tile([128, 4, N_HEADS, D_K + 1], f32, tag="pc")
                        for s in range(4):
                            for h in range(N_HEADS):
                                nc.tensor.matmul(
                                    out=ctx_ps[:, s, h, :],
                                    lhsT=at_q[qi][:, s, h, :],
                                    rhs=vp[:, j0 + s, h, :],
                                    start=True, stop=True,
                                )
                        recip = work.tile([128, 4, N_HEADS, 1], f32, tag="recip",
                                          bufs=4)
                        nc.vector.reciprocal(
                            out=recip[:], in_=ctx_ps[:, :, :, D_K : D_K + 1]
                        )
                        nc.vector.tensor_mul(
                            out=ctx_g[:, j0 : j0 + 4, :].rearrange(
                                "p s (h e) -> p s h e", h=N_HEADS
                            ),
                            in0=ctx_ps[:, :, :, 0:D_K],
                            in1=recip[:].to_broadcast([128, 4, N_HEADS, D_K]),
                        )

                    # ---- E: ctx^T -> Wo -> +x -> LN1 -> x2
                    ctxt = work.tile([64, G, 128], f32, tag="ctxt")
                    for qj in range(G // 4):
                        tp = pa.tile([64, 4, 128], f32, tag="pab")
                        for s in range(4):
                            nc.tensor.transpose(
                                out=tp[:, s, :], in_=ctx_g[:, 4 * qj + s, :],
                                identity=ident[:],
                            )
                        nc.vector.tensor_copy(
                            out=ctxt[:, 4 * qj : 4 * qj + 4, :], in_=tp[:]
                        )
                    v1_g = work.tile([128, G, D_MODEL], f32, tag="v1")
                    for qi in range(NQUAD):
                        j0 = 4 * qi
                        ao_ps = pc.tile([128, 4, D_MODEL], f32, tag="pc")
                        for s in range(4):
                            nc.tensor.matmul(
                                out=ao_ps[:, s, :], lhsT=ctxt[:, j0 + s, :],
                                rhs=wo_sb[:, layer, :], start=True, stop=True,
                            )
                        nc.vector.tensor_add(
                            out=v1_g[:, j0 : j0 + 4, :], in0=ao_ps[:],
                            in1=x_g[:, j0 : j0 + 4, :],
                        )
                    x2_g = work.tile([128, G, D_MODEL], f32, tag="x2")
                    for qi in range(NQUAD):
                        _ln_quad(nc, work, eps_t, v1_g,
                                 lambda j: x2_g[:, j, :], 4 * qi, "a", mybir)

                    # ---- F: x2^T -> W1+relu -> W2 -> +x2 -> LN2 -> x
                    x2t = work.tile([64, G, 128], bf16, tag="x2t")
                    for qj in range(G // 4):
                        tp = pa.tile([64, 4, 128], f32, tag="pab")
                        for s in range(4):
                            nc.tensor.transpose(
                                out=tp[:, s, :], in_=x2_g[:, 4 * qj + s, :],
                                identity=ident[:],
                            )
                        nc.vector.tensor_copy(
                            out=x2t[:, 4 * qj : 4 * qj + 4, :], in_=tp[:]
                        )
                    ht = work.tile([128, 4, NQUAD, 4, 128], bf16, tag="ht", bufs=1)
                    for qi in range(NQUAD):
                        j0 = 4 * qi
                        for c in range(4):
                            h_ps = pa.tile([128, 512], f32, tag="pa")
                            nc.tensor.matmul(
                                out=h_ps[:],
                                lhsT=w1_sb[:, layer, 128 * c : 128 * (c + 1)],
                                rhs=x2t[:, j0 : j0 + 4, :], start=True, stop=True,
                            )
                            nc.scalar.activation(
                                out=ht[:, c, qi, :, :].rearrange("p s t -> p (s t)"),
                                in_=h_ps[:], func=AF.Relu,
                            )
                    v2_g = work.tile([128, G, D_MODEL], f32, tag="v2")
                    for qi in range(NQUAD):
                        j0 = 4 * qi
                        y_ps = pc.tile([128, 4, D_MODEL], f32, tag="pc")
                        for s in range(4):
                            for c in range(4):
                                nc.tensor.matmul(
                                    out=y_ps[:, s, :], lhsT=ht[:, c, qi, s, :],
                                    rhs=w2_sb[:, layer, c, :],
                                    start=(c == 0), stop=(c == 3),
                                )
                        nc.vector.tensor_add(
                            out=v2_g[:, j0 : j0 + 4, :], in0=y_ps[:],
                            in1=x2_g[:, j0 : j0 + 4, :],
                        )
                    for qi in range(NQUAD):
                        _ln_quad(nc, work, eps_t, v2_g,
                                 lambda j, _x=x_g: _x[:, j, :], 4 * qi, "b", mybir)

            for gi in range(n_groups):
                nc.sync.dma_start(
                    out=out_d[gi * G : (gi + 1) * G].rearrange("b l d -> l b d"),
                    in_=xs[gi][:],
                )

    _split_multi_waits(nc)
    return nc


def _ln_quad(nc, work, eps_t, v_g, out_ap_of, j0, tag, mybir):
    """Per-quad LN over free dim 64: batches j0..j0+3 of v_g [128, G, 64]."""
    f32 = mybir.dt.float32
    AF = mybir.ActivationFunctionType
    ALU = mybir.AluOpType
    Q = 4
    stats = work.tile([128, Q, 6], f32, tag=f"lns{tag}", bufs=8)
    mv = work.tile([128, Q, 2], f32, tag=f"lnm{tag}", bufs=8)
    for s in range(Q):
        nc.vector.bn_stats(out=stats[:, s, :], in_=v_g[:, j0 + s, :])
        nc.vector.bn_aggr(out=mv[:, s, :], in_=stats[:, s, :])
    # rstd = exp(-0.5*ln(var+eps)): Ln/Exp share one ACT table set with
    # Relu/Copy/Identity, so no act-table swaps anywhere in the kernel
    # (Sqrt lives in a different set; each swap costs ~1.3us).
    lv = work.tile([128, Q, 1], f32, tag=f"lnstd{tag}", bufs=8)
    nc.scalar.activation(
        out=lv[:], in_=mv[:, :, 1:2], func=AF.Ln, bias=eps_t[:, 0:1], scale=1.0
    )
    rstd = work.tile([128, Q, 1], f32, tag=f"lnr{tag}", bufs=8)
    nc.scalar.activation(
        out=rstd[:], in_=lv[:], func=AF.Exp, bias=0.0, scale=-0.5
    )
    nmr = work.tile([128, Q, 1], f32, tag=f"lnn{tag}", bufs=8)
    # scalar_tensor_tensor wedges the device on this runtime - use 2 ops
    nc.vector.tensor_mul(out=nmr[:], in0=mv[:, :, 0:1], in1=rstd[:])
    nc.vector.tensor_scalar_mul(nmr[:], nmr[:], -1.0)
    aeng = nc.gpsimd
    for s in range(Q):
        aeng.tensor_scalar(
            out=out_ap_of(j0 + s), in0=v_g[:, j0 + s, :],
            scalar1=rstd[:, s, 0:1], scalar2=nmr[:, s, 0:1],
            op0=ALU.mult, op1=ALU.add,
        )



_NC_CACHE = {}


def run(inputs, trace=False, **spmd_kwargs):
    from concourse.bass_utils import run_bass_kernel_spmd

    x0, ebt, wq, wk, wv, wo, w1, w2 = _host_prep(inputs)

    if "nc" not in _NC_CACHE:
        _NC_CACHE["nc"] = build_nc()
    nc = _NC_CACHE["nc"]

    in_maps = []
    for c in range(N_CORES):
        sl = slice(c * B_LOC, (c + 1) * B_LOC)
        in_maps.append(
            dict(
                x0=np.ascontiguousarray(x0[sl]),
                ebt=np.ascontiguousarray(ebt[sl]),
                wq=wq, wk=wk, wv=wv, wo=wo, w1=w1, w2=w2,
            )
        )

    res = run_bass_kernel_spmd(
        nc, in_maps, core_ids=list(range(N_CORES)), trace=trace, **spmd_kwargs
    )
    out = np.concatenate(
        [np.asarray(res.results[c]["out"]) for c in range(N_CORES)], axis=0
    )
    return out.astype(np.float32), res


def kernel(**inputs):
    out, _ = run(inputs)
    return out


def _jit_single_core(nc):
    """Build a single-device jitted callable for nc (same program as SPMD)."""
    import jax
    from concourse import bass2jax
    from concourse import mybir

    bass2jax.install_neuronx_cc_hook()
    in_names, out_names, out_avals, zero_outs = [], [], [], []
    partition_name = nc.partition_id_tensor.name if nc.partition_id_tensor else None
    for alloc in nc.m.functions[0].allocations:
        if not isinstance(alloc, mybir.MemoryLocationSet):
            continue
        name = alloc.memorylocations[0].name
        if alloc.kind == "ExternalInput":
            if name != partition_name:
                in_names.append(name)
        elif alloc.kind == "ExternalOutput":
            out_names.append(name)
            shape = tuple(alloc.tensor_shape)
            dtype = mybir.dt.np(alloc.dtype)
            out_avals.append(jax.core.ShapedArray(shape, dtype))
            zero_outs.append(np.zeros(shape, dtype))
    n_params = len(in_names)
    all_names = in_names + out_names + ([partition_name] if partition_name else [])
    donate = tuple(range(n_params, n_params + len(out_names)))

    def _body(*args):
        operands = list(args)
        if partition_name is not None:
            operands.append(bass2jax.partition_id_tensor())
        outs = bass2jax._bass_exec_p.bind(
            *operands,
            out_avals=tuple(out_avals),
            in_names=tuple(all_names),
            out_names=tuple(out_names),
            lowering_input_output_aliases=(),
            sim_require_finite=True,
            sim_require_nnan=True,
            nc=nc,
        )
        return tuple(outs)

    jfn = jax.jit(_body, donate_argnums=donate, keep_unused=True)
    return jfn, in_names, zero_outs


def bench_marginal(inputs, iters=24, reps=2):
    """Per-execution device time via async dispatch pipelining: issue
    `iters` executions without blocking (independent submissions pipeline on
    the core), block once at the end; marginal over 1-call runs cancels the
    ~90 ms axon dispatch overhead."""
    import time

    import jax

    x0, ebt, wq, wk, wv, wo, w1, w2 = _host_prep(inputs)
    if "nc" not in _NC_CACHE:
        _NC_CACHE["nc"] = build_nc()
    nc = _NC_CACHE["nc"]
    in_map = dict(
        x0=np.ascontiguousarray(x0[:B_LOC]),
        ebt=np.ascontiguousarray(ebt[:B_LOC]),
        wq=wq, wk=wk, wv=wv, wo=wo, w1=w1, w2=w2,
    )
    jfn, in_names, zero_outs = _jit_single_core(nc)
    dev = jax.devices()[0]
    ins_dev = [jax.device_put(np.asarray(in_map[n]), dev) for n in in_names]
    n_zsets = (iters + 2) * reps + 4
    zsets = [
        [jax.device_put(z.copy(), dev) for z in zero_outs] for _ in range(n_zsets)
    ]
    jax.block_until_ready(zsets)
    jax.block_until_ready(ins_dev)
    state = {"zi": 0}

    def run_m(m):
        outs = []
        t0 = time.perf_counter()
        for _ in range(m):
            outs.append(jfn(*ins_dev, *zsets[state["zi"]]))
            state["zi"] += 1
        jax.block_until_ready(outs)
        return time.perf_counter() - t0

    run_m(1)  # warm (compiles)
    t1s, tns = [], []
    for _ in range(reps):
        t1s.append(run_m(1))
        tns.append(run_m(iters))
    marginal_ns = (min(tns) - min(t1s)) / (iters - 1) * 1e9
    return dict(
        est_exec_ns=marginal_ns,
        t1_ns=min(t1s) * 1e9,
        tn_ns=min(tns) * 1e9,
        t1s=t1s,
        tns=tns,
        iters=iters,
    )





# revision 19
# speedup vs baseline: 3.2970x; 3.2970x over previous
"""Trainium2 Bass kernel v4 for nn_Encoder_88656714924838.

6-layer encoder, d_model=64, 4 heads x dk=16, d_ff=512, B=256, L=128.
Data parallel over 8 cores (32 batches/core). Device kernel does all layers.

v4 = v3 design with the HW constraint found by bisection: matmuls with
different operand base partitions (0 vs 64) must NOT share a PSUM tile
(same-bank base mixing aborts the device; grouped-by-base tiles are fine).
All batch loops are parity-major: slot (p, j) <-> batch b = 2j + p; every
PSUM tile receives 4 same-parity matmuls. Heads are processed in order
(0,2,1,3) so score tiles split by head parity too; host reorders ebt heads
and Wo rows to match.

- All-bf16 matmul operands (fp32 matmul is 4 cyc/col vs 1 on TRN2 PE);
  f32 residual stream for accuracy.
- k^T duplicated on both partition halves ([Wk|Wk] lhsT) + 2-head-packed
  padded Wq -> scores run as 4 x [K=64,M=128,N=128] per batch with operands
  at base 0 (even heads) / base 64 (odd heads). No DMA shuffles (HWDGE DMA
  is ~1.6us/op serialized - too slow for inner loops).
- bf16/f32 2-batch pair transposes ([128=(2x64d), pair, 128] layout) with
  weights duplicated on both halves for the base-64 parity.
- Pool cannot touch PSUM; DMA cannot touch PSUM. Evacs split Act/DVE,
  at-mul and LN applies on Pool, relu split Act/DVE.
"""

import sys

for _p in ("/opt/trn_rl_repo",):
    if _p not in sys.path:
        sys.path.insert(0, _p)

import numpy as np

D_MODEL = 64
N_HEADS = 4
D_K = 16
D_FF = 512
N_LAYERS = 6
B, L = 256, 128
N_CORES = 8
B_LOC = B // N_CORES
SCALE = 1.0 / np.sqrt(np.float32(D_K))

G = 16  # batches per group
NPAIR = G // 2
NQUAD = G // 4
HR = (0, 2, 1, 3)  # head processing order (parity-major)


def _positional_encoding(length=L, d_model=D_MODEL):
    pos = np.arange(length, dtype=np.float32)[:, None]
    div = np.exp(
        np.arange(0, d_model, 2, dtype=np.float32) * (-np.log(10000.0) / d_model)
    )
    pe = np.zeros((length, d_model), dtype=np.float32)
    pe[:, 0::2] = np.sin(pos * div)
    pe[:, 1::2] = np.cos(pos * div)
    return pe


def _split_multi_waits(nc):
    """walrus accepts only ONE sync-wait per instruction; hoist extras onto
    same-engine NoOps just before the carrying instruction."""
    import concourse.mybir as mybir

    k = 0
    for fn in nc.m.functions:
        for blk in fn.blocks:
            new = []
            changed = False
            for inst in blk.instructions:
                si = inst.sync_info
                waits = list(si.on_wait) if (si and si.on_wait) else []
                if len(waits) > 1:
                    changed = True
                    for w in waits[:-1]:
                        k += 1
                        nop = mybir.InstNoOp(name=f"ws-{k}", ins=[], outs=[])
                        nop.engine = inst.engine
                        nop.sync_info = mybir.SyncInfo(on_wait=[w], on_update=[])
                        nc.register_instruction(nop)
                        new.append(nop)
                    si.on_wait = waits[-1:]
                new.append(inst)
            if changed:
                blk.instructions = new


def _host_prep(inputs):
    import ml_dtypes

    bf = ml_dtypes.bfloat16
    enc = np.asarray(inputs["enc_inputs"])
    deg = np.asarray(inputs["degree_s"])
    MD = np.asarray(inputs["MD"])
    src_emb = np.asarray(inputs["src_emb"], dtype=np.float32)
    deg_emb = np.asarray(inputs["deg_emb"], dtype=np.float32)
    md_emb = np.asarray(inputs["md_emb"], dtype=np.float32)

    x0 = (src_emb[enc] + deg_emb[deg] + _positional_encoding()[None]).astype(
        np.float32
    )

    # x0^T in 2-batch pair layout: [B/2, 128=(d of even | d of odd), 128=L]
    x0t = np.ascontiguousarray(x0.transpose(0, 2, 1))  # [B, 64, 128]
    x0tp = x0t.reshape(B // 2, 2 * D_MODEL, L).astype(bf)

    # scores^T layout [b, key, hr, query] with heads reordered (0,2,1,3);
    # fold key pad-mask; exponentiate.
    bias_t = np.ascontiguousarray(md_emb[MD].transpose(0, 2, 3, 1))  # [b,k,q,h]->
    # -> [b, key, h, query]? md_emb[MD] is [b, q, k, h]; transpose to [b,k,h,q]
    bias_t = np.ascontiguousarray(md_emb[MD].transpose(0, 2, 3, 1))
    mask = np.where(enc == 0, np.float32(-1e9), np.float32(0.0))
    with np.errstate(under="ignore"):
        ebt = np.exp(bias_t + mask[:, :, None, None], dtype=np.float32)
    ebt = np.ascontiguousarray(ebt[:, :, HR, :]).astype(bf)

    def dup(w):  # [n, 64, m] -> [128, n, m] rows 0:64 == rows 64:128
        w2 = np.concatenate([w, w], axis=1)  # [n, 128, m]
        return np.ascontiguousarray(w2.transpose(1, 0, 2)).astype(bf)

    wq = np.asarray(inputs["Wq"], dtype=np.float32) * SCALE
    wk = np.asarray(inputs["Wk"], dtype=np.float32)
    # k^T duplicated on both output halves
    wkk_d = dup(np.concatenate([wk, wk], axis=2))  # [128, n, 128]
    # 2-head-packed padded Wq: j holds heads (HR[2j], HR[2j+1]) = ((0,2),(1,3))
    wqp = np.zeros((N_LAYERS, D_MODEL, 2, 2, D_MODEL), dtype=np.float32)
    for hs in range(N_HEADS):
        h = HR[hs]
        sl = slice(D_K * h, D_K * (h + 1))
        wqp[:, :, hs % 2, hs // 2, sl] = wq[:, :, sl]
    # wqp[:,:,j,s,:]: j = head parity (0: heads 0,2; 1: heads 1,3), s = slot
    wqp = wqp.reshape(N_LAYERS, D_MODEL, 2, 128)
    wqp_d = np.ascontiguousarray(
        np.concatenate([wqp, wqp], axis=1).transpose(1, 0, 2, 3)
    ).astype(bf)  # [128, n, 2, 128]
    wv_d = dup(np.asarray(inputs["Wv"], dtype=np.float32))  # [128, n, 64]
    # Wo rows reordered to (hr, e) blocks to match ctx layout
    wo = np.asarray(inputs["Wo"], dtype=np.float32)  # [n, 64, 64]
    wo_r = wo.reshape(N_LAYERS, N_HEADS, D_K, D_MODEL)[:, HR, :, :].reshape(
        N_LAYERS, D_MODEL, D_MODEL
    )
    wo_d = dup(wo_r)  # [128, n, 64]
    w1_d = dup(np.asarray(inputs["W1"], dtype=np.float32))  # [128, n, 512]
    w2_ = np.ascontiguousarray(
        np.asarray(inputs["W2"], dtype=np.float32)
        .reshape(N_LAYERS, 4, 128, D_MODEL)
        .transpose(2, 0, 1, 3)
    ).astype(bf)
    return x0, x0tp, ebt, wkk_d, wqp_d, wv_d, wo_d, w1_d, w2_


def build_nc(n_layers=N_LAYERS, b_loc=B_LOC):
    import concourse.bass as bass
    import concourse.mybir as mybir
    import concourse.tile as tile
    from concourse.masks import make_identity

    f32 = mybir.dt.float32
    bf16 = mybir.dt.bfloat16
    AF = mybir.ActivationFunctionType

    nc = bass.Bass("TRN2", target_bir_lowering=False, debug=False)

    x0_d = nc.dram_tensor("x0", [b_loc, L, D_MODEL], f32, kind="ExternalInput")
    x0t_d = nc.dram_tensor("x0t", [b_loc // 2, 128, L], bf16, kind="ExternalInput")
    ebt_d = nc.dram_tensor("ebt", [b_loc, L, N_HEADS, L], bf16, kind="ExternalInput")
    wkk_d = nc.dram_tensor("wkk", [128, n_layers, 128], bf16, kind="ExternalInput")
    wqp_d = nc.dram_tensor("wqp", [128, n_layers, 2, 128], bf16,
                           kind="ExternalInput")
    wv_d = nc.dram_tensor("wv", [128, n_layers, D_MODEL], bf16, kind="ExternalInput")
    wo_d = nc.dram_tensor("wo", [128, n_layers, D_MODEL], bf16, kind="ExternalInput")
    w1_d = nc.dram_tensor("w1", [128, n_layers, D_FF], bf16, kind="ExternalInput")
    w2_d = nc.dram_tensor("w2", [128, n_layers, 4, D_MODEL], bf16,
                          kind="ExternalInput")
    out_d = nc.dram_tensor("out", [b_loc, L, D_MODEL], f32, kind="ExternalOutput")

    n_groups = b_loc // G

    with tile.TileContext(nc) as tc:
        with (
            tc.tile_pool(name="consts", bufs=1) as consts,
            tc.tile_pool(name="state", bufs=1) as state,
            tc.tile_pool(name="work", bufs=2) as work,
            tc.tile_pool(name="pa", bufs=2, space="PSUM") as pa,
            tc.tile_pool(name="pst", bufs=2, space="PSUM") as pst,
            tc.tile_pool(name="pc", bufs=2, space="PSUM") as pc,
        ):
            ident = consts.tile([128, 128], f32)
            make_identity(nc, ident[:])
            eps_t = consts.tile([128, 1], f32)
            nc.vector.memset(eps_t[:], 1e-5)

            wkk_sb = consts.tile([128, n_layers, 128], bf16)
            nc.sync.dma_start(out=wkk_sb[:], in_=wkk_d.ap())
            wqp_sb = consts.tile([128, n_layers, 2, 128], bf16)
            nc.sync.dma_start(out=wqp_sb[:], in_=wqp_d.ap())
            wv_sb = consts.tile([128, n_layers, D_MODEL], bf16)
            nc.sync.dma_start(out=wv_sb[:], in_=wv_d.ap())
            wo_sb = consts.tile([128, n_layers, D_MODEL], bf16)
            nc.sync.dma_start(out=wo_sb[:], in_=wo_d.ap())
            w1_sb = consts.tile([128, n_layers, D_FF], bf16)
            nc.sync.dma_start(out=w1_sb[:], in_=w1_d.ap())
            w2_sb = consts.tile([128, n_layers, 4, D_MODEL], bf16)
            nc.sync.dma_start(out=w2_sb[:], in_=w2_d.ap())

            # state: parity-major [128, parity, pair, ...]; batch b = 2j + p
            xs, xts, vps = [], [], []
            for gi in range(n_groups):
                xg = state.tile([128, NPAIR, 2, D_MODEL], f32, tag=f"x{gi}")
                nc.sync.dma_start(
                    out=xg[:].rearrange("l j p d -> l (j p) d"),
                    in_=x0_d[gi * G : (gi + 1) * G].rearrange("b l d -> l b d"),
                )
                xs.append(xg)
                xt = state.tile([128, NPAIR, L], bf16, tag=f"xt{gi}")
                nc.sync.dma_start(
                    out=xt[:],
                    in_=x0t_d[gi * NPAIR : (gi + 1) * NPAIR].rearrange(
                        "p k t -> k p t"
                    ),
                )
                xts.append(xt)
                vg = state.tile([128, NPAIR, 2, N_HEADS, D_K + 1], bf16,
                                tag=f"vp{gi}")
                nc.vector.memset(vg[:, :, :, :, D_K : D_K + 1], 1.0)
                vps.append(vg)
            # eb parity-major slots: [128, slot(=p*16+...)]: use [128, 2, 16, 4, 128]
            # per group-half? simpler: [128, n_groups, 2, NPAIR, 4, 128]
            eb_sb = state.tile([128, n_groups, NPAIR, 2, N_HEADS, L], bf16)
            for b in range(b_loc):
                gi, bl = b // G, b % G
                nc.sync.dma_start(
                    out=eb_sb[:, gi, bl // 2, bl % 2, :, :], in_=ebt_d[b]
                )

            for layer in range(n_layers):
                for g in range(n_groups):
                    x_g = xs[g]
                    xt_g = xts[g]
                    vp = vps[g]

                    # ---- A: k^T-dup + padded q^T + V, parity-grouped quads
                    kt = work.tile([128, 2, NPAIR, 128], bf16, tag="kt", bufs=2)
                    qp = work.tile([128, 2, 2, NPAIR, 128], bf16, tag="qp", bufs=2)
                    for sq in range(4):
                        p, pr0 = sq // 2, 4 * (sq % 2)
                        b64 = 64 * p
                        kt_ps = pa.tile([128, 4, 128], f32, tag="pa")
                        for i in range(4):
                            nc.tensor.matmul(
                                out=kt_ps[:, i, :],
                                lhsT=wkk_sb[b64 : b64 + 64, layer, :],
                                rhs=xt_g[b64 : b64 + 64, pr0 + i, :],
                                start=True, stop=True,
                            )
                        qeng = (nc.scalar.copy, nc.vector.tensor_copy)[sq % 2]
                        qeng(out=kt[:, p, pr0 : pr0 + 4, :], in_=kt_ps[:])
                        for j in range(2):
                            qp_ps = pa.tile([128, 4, 128], f32, tag="pa")
                            for i in range(4):
                                nc.tensor.matmul(
                                    out=qp_ps[:, i, :],
                                    lhsT=wqp_sb[b64 : b64 + 64, layer, j, :],
                                    rhs=xt_g[b64 : b64 + 64, pr0 + i, :],
                                    start=True, stop=True,
                                )
                            qeng2 = (nc.scalar.copy, nc.vector.tensor_copy)[j]
                            qeng2(
                                out=qp[:, j, p, pr0 : pr0 + 4, :], in_=qp_ps[:]
                            )
                        v_ps = pc.tile([128, 4, 68], f32, tag="pc")
                        for i in range(4):
                            nc.tensor.matmul(
                                out=v_ps[:, i, 0:D_MODEL],
                                lhsT=xt_g[b64 : b64 + 64, pr0 + i, :],
                                rhs=wv_sb[b64 : b64 + 64, layer, :],
                                start=True, stop=True,
                            )
                        nc.scalar.copy(
                            out=vp[:, pr0 : pr0 + 4, p, :, 0:D_K],
                            in_=v_ps[:, :, 0:D_MODEL].rearrange(
                                "p s (h e) -> p s h e", h=N_HEADS
                            ),
                        )

                    # ---- D/E: scores -> exp -> *exp(bias) -> ctx -> normalize
                    # hslot order HR=(0,2,1,3): st_e holds hslots 0,1 (base 0),
                    # st_o hslots 2,3 (base 64)
                    ctx_g = work.tile([128, NPAIR, 2, D_MODEL], f32, tag="ctx")
                    for p in range(2):
                        ats = {}
                        for bb in range(0, NPAIR, 2):
                            for par, b64 in ((0, 0), (1, 64)):
                                st_ps = pst.tile(
                                    [128, 2, 2, 128], f32, tag=f"pst{par}"
                                )
                                for ii in range(2):
                                    for js in range(2):
                                        nc.tensor.matmul(
                                            out=st_ps[:, ii, js, :],
                                            lhsT=kt[b64 : b64 + 64, p, bb + ii, :],
                                            rhs=qp[b64 : b64 + 64, js, p,
                                                   bb + ii, :],
                                            start=True, stop=True,
                                        )
                                ex = work.tile([128, 2, 2, 128], bf16,
                                               tag=f"ex{par}", bufs=4)
                                nc.scalar.activation(
                                    out=ex[:], in_=st_ps[:], func=AF.Exp
                                )
                                at = work.tile([128, 2, 2, 128], bf16,
                                               tag=f"at{par}", bufs=4)
                                nc.gpsimd.tensor_mul(
                                    out=at[:], in0=ex[:],
                                    in1=eb_sb[:, g, bb : bb + 2, p,
                                              2 * par : 2 * par + 2, :],
                                )
                                ats[(bb, par)] = at
                            if bb % 4 == 2:
                                pr0 = bb - 2
                                ctx_ps = pc.tile([128, 4, 68], f32, tag="pc")
                                ctx_v = ctx_ps[:].rearrange(
                                    "p s (h e) -> p s h e", h=4
                                )
                                for i in range(4):
                                    pr = pr0 + i
                                    for hs in range(4):
                                        par, js = hs // 2, hs % 2
                                        nc.tensor.matmul(
                                            out=ctx_v[:, i, hs, :],
                                            lhsT=ats[(pr0 + 2 * (i // 2), par)][
                                                :, i % 2, js, :
                                            ],
                                            rhs=vp[:, pr, p, HR[hs], :],
                                            start=True, stop=True,
                                        )
                                recip = work.tile([128, 4, 4, 1], f32,
                                                  tag="recip", bufs=4)
                                nc.vector.reciprocal(
                                    out=recip[:],
                                    in_=ctx_v[:, :, :, D_K : D_K + 1],
                                )
                                nc.vector.tensor_mul(
                                    out=ctx_g[:, pr0 : pr0 + 4, p, :].rearrange(
                                        "p s (h e) -> p s h e", h=4
                                    ),
                                    in0=ctx_v[:, :, :, 0:D_K],
                                    in1=recip[:].to_broadcast([128, 4, 4, D_K]),
                                )

                    # ---- F: ctx^T pairs -> Wo -> +x -> LN1 -> x2
                    ctxt = work.tile([128, NPAIR, 128], bf16, tag="ctxt")
                    for tj in range(2):
                        tp = pa.tile([128, 4, 128], f32, tag="pa")
                        for pj in range(4):
                            nc.tensor.transpose(
                                out=tp[:, pj, :],
                                in_=ctx_g[:, 4 * tj + pj, :, :],
                                identity=ident[:],
                            )
                        teng = (nc.scalar.copy, nc.vector.tensor_copy)[tj]
                        teng(out=ctxt[:, 4 * tj : 4 * tj + 4, :], in_=tp[:])
                    v1_g = work.tile([128, NPAIR, 2, D_MODEL], f32, tag="v1")
                    for sq in range(4):
                        p, pr0 = sq // 2, 4 * (sq % 2)
                        b64 = 64 * p
                        ao_ps = pc.tile([128, 4, 68], f32, tag="pc")
                        for i in range(4):
                            nc.tensor.matmul(
                                out=ao_ps[:, i, 0:D_MODEL],
                                lhsT=ctxt[b64 : b64 + 64, pr0 + i, :],
                                rhs=wo_sb[b64 : b64 + 64, layer, :],
                                start=True, stop=True,
                            )
                        nc.vector.tensor_add(
                            out=v1_g[:, pr0 : pr0 + 4, p, :],
                            in0=ao_ps[:, :, 0:D_MODEL],
                            in1=x_g[:, pr0 : pr0 + 4, p, :],
                        )
                    x2_g = work.tile([128, NPAIR, 2, D_MODEL], f32, tag="x2")
                    _ln_group(nc, work, eps_t, v1_g, x2_g, "a", mybir)

                    # ---- G: x2^T pairs -> W1+relu -> W2 -> +x2 -> LN2 -> x
                    x2t = work.tile([128, NPAIR, 128], bf16, tag="x2t")
                    for tj in range(2):
                        tp = pa.tile([128, 4, 128], f32, tag="pa")
                        for pj in range(4):
                            nc.tensor.transpose(
                                out=tp[:, pj, :],
                                in_=x2_g[:, 4 * tj + pj, :, :],
                                identity=ident[:],
                            )
                        teng = (nc.scalar.copy, nc.vector.tensor_copy)[tj]
                        teng(out=x2t[:, 4 * tj : 4 * tj + 4, :], in_=tp[:])
                    v2_g = work.tile([128, NPAIR, 2, D_MODEL], f32, tag="v2")
                    for sq in range(4):
                        p, pr0 = sq // 2, 4 * (sq % 2)
                        b64 = 64 * p
                        ht = work.tile([128, 4, 4, 128], bf16, tag="ht", bufs=3)
                        for c in range(4):
                            h_ps = pa.tile([128, 4, 128], f32, tag="pa")
                            for i in range(4):
                                nc.tensor.matmul(
                                    out=h_ps[:, i, :],
                                    lhsT=w1_sb[b64 : b64 + 64, layer,
                                               128 * c : 128 * (c + 1)],
                                    rhs=x2t[b64 : b64 + 64, pr0 + i, :],
                                    start=True, stop=True,
                                )
                            if c % 2 == 0:
                                nc.scalar.activation(
                                    out=ht[:, c, :, :], in_=h_ps[:], func=AF.Relu
                                )
                            else:
                                nc.vector.tensor_scalar_max(
                                    ht[:, c, :, :], h_ps[:], 0.0
                                )
                        y_ps = pc.tile([128, 4, 68], f32, tag="pc")
                        for i in range(4):
                            for c in range(4):
                                nc.tensor.matmul(
                                    out=y_ps[:, i, 0:D_MODEL],
                                    lhsT=ht[:, c, i, :],
                                    rhs=w2_sb[:, layer, c, :],
                                    start=(c == 0), stop=(c == 3),
                                )
                        nc.vector.tensor_add(
                            out=v2_g[:, pr0 : pr0 + 4, p, :],
                            in0=y_ps[:, :, 0:D_MODEL],
                            in1=x2_g[:, pr0 : pr0 + 4, p, :],
                        )
                    _ln_group(nc, work, eps_t, v2_g, x_g, "b", mybir)
                    if layer < n_layers - 1:
                        for tj in range(2):
                            tp = pa.tile([128, 4, 128], f32, tag="pa")
                            for pj in range(4):
                                nc.tensor.transpose(
                                    out=tp[:, pj, :],
                                    in_=x_g[:, 4 * tj + pj, :, :],
                                    identity=ident[:],
                                )
                            teng = (nc.scalar.copy, nc.vector.tensor_copy)[tj]
                            teng(out=xt_g[:, 4 * tj : 4 * tj + 4, :], in_=tp[:])

            for gi in range(n_groups):
                nc.sync.dma_start(
                    out=out_d[gi * G : (gi + 1) * G].rearrange("b l d -> l b d"),
                    in_=xs[gi][:].rearrange("l j p d -> l (j p) d"),
                )

    _split_multi_waits(nc)
    return nc


def _ln_group(nc, work, eps_t, v_g, out_g, tag, mybir):
    """LN over free dim 64 for v_g [128, 2, 8, 64] f32 -> out_g same shape."""
    f32 = mybir.dt.float32
    AF = mybir.ActivationFunctionType
    ALU = mybir.AluOpType
    stats = work.tile([128, NPAIR, 2, 6], f32, tag=f"lns{tag}", bufs=2)
    mv = work.tile([128, NPAIR, 2, 2], f32, tag=f"lnm{tag}", bufs=2)
    for j in range(NPAIR):
        for p in range(2):
            nc.vector.bn_stats(out=stats[:, j, p, :], in_=v_g[:, j, p, :])
            nc.vector.bn_aggr(out=mv[:, j, p, :], in_=stats[:, j, p, :])
    # rstd = exp(-0.5*ln(var+eps)); Ln/Exp share the ACT table set with
    # Relu/Copy/Identity so there are no act-table swaps in the kernel.
    lv = work.tile([128, NPAIR, 2, 1], f32, tag=f"lnstd{tag}", bufs=2)
    nc.scalar.activation(
        out=lv[:], in_=mv[:, :, :, 1:2], func=AF.Ln, bias=eps_t[:, 0:1], scale=1.0
    )
    rstd = work.tile([128, NPAIR, 2, 1], f32, tag=f"lnr{tag}", bufs=2)
    nc.scalar.activation(out=rstd[:], in_=lv[:], func=AF.Exp, bias=0.0, scale=-0.5)
    nmr = work.tile([128, NPAIR, 2, 1], f32, tag=f"lnn{tag}", bufs=2)
    # scalar_tensor_tensor wedges the device on this runtime - use 2 ops
    nc.gpsimd.tensor_mul(out=nmr[:], in0=mv[:, :, :, 0:1], in1=rstd[:])
    nc.gpsimd.tensor_scalar_mul(nmr[:], nmr[:], -1.0)
    for j in range(NPAIR):
        for p in range(2):
            nc.gpsimd.tensor_scalar(
                out=out_g[:, j, p, :], in0=v_g[:, j, p, :],
                scalar1=rstd[:, j, p, 0:1], scalar2=nmr[:, j, p, 0:1],
                op0=ALU.mult, op1=ALU.add,
            )


_NC_CACHE = {}


def run(inputs, trace=False, **spmd_kwargs):
    from concourse.bass_utils import run_bass_kernel_spmd

    x0, x0tp, ebt, wkk, wqp, wv, wo, w1, w2 = _host_prep(inputs)

    if "nc" not in _NC_CACHE:
        _NC_CACHE["nc"] = build_nc()
    nc = _NC_CACHE["nc"]

    in_maps = []
    for c in range(N_CORES):
        sl = slice(c * B_LOC, (c + 1) * B_LOC)
        slp = slice(c * B_LOC // 2, (c + 1) * B_LOC // 2)
        in_maps.append(
            dict(
                x0=np.ascontiguousarray(x0[sl]),
                x0t=np.ascontiguousarray(x0tp[slp]),
                ebt=np.ascontiguousarray(ebt[sl]),
                wkk=wkk, wqp=wqp, wv=wv, wo=wo, w1=w1, w2=w2,
            )
        )

    res = run_bass_kernel_spmd(
        nc, in_maps, core_ids=list(range(N_CORES)), trace=trace, **spmd_kwargs
    )
    out = np.concatenate(
        [np.asarray(res.results[c]["out"]) for c in range(N_CORES)], axis=0
    )
    return out.astype(np.float32), res


def kernel(**inputs):
    out, _ = run(inputs)
    return out


_PERM = np.concatenate(
    [g * G + np.array([2 * j + p for p in range(2) for j in range(NPAIR)])
     for g in range(B // G)]
)
_IPERM = np.argsort(_PERM)


def _jit_single_core(nc):
    """Build a single-device jitted callable for nc (same program as SPMD)."""
    import jax
    from concourse import bass2jax
    from concourse import mybir

    bass2jax.install_neuronx_cc_hook()
    in_names, out_names, out_avals, zero_outs = [], [], [], []
    partition_name = nc.partition_id_tensor.name if nc.partition_id_tensor else None
    for alloc in nc.m.functions[0].allocations:
        if not isinstance(alloc, mybir.MemoryLocationSet):
            continue
        name = alloc.memorylocations[0].name
        if alloc.kind == "ExternalInput":
            if name != partition_name:
                in_names.append(name)
        elif alloc.kind == "ExternalOutput":
            out_names.append(name)
            shape = tuple(alloc.tensor_shape)
            dtype = mybir.dt.np(alloc.dtype)
            out_avals.append(jax.core.ShapedArray(shape, dtype))
            zero_outs.append(np.zeros(shape, dtype))
    n_params = len(in_names)
    all_names = in_names + out_names + ([partition_name] if partition_name else [])
    donate = tuple(range(n_params, n_params + len(out_names)))

    def _body(*args):
        operands = list(args)
        if partition_name is not None:
            operands.append(bass2jax.partition_id_tensor())
        outs = bass2jax._bass_exec_p.bind(
            *operands,
            out_avals=tuple(out_avals),
            in_names=tuple(all_names),
            out_names=tuple(out_names),
            lowering_input_output_aliases=(),
            sim_require_finite=True,
            sim_require_nnan=True,
            nc=nc,
        )
        return tuple(outs)

    jfn = jax.jit(_body, donate_argnums=donate, keep_unused=True)
    return jfn, in_names, zero_outs


def bench_marginal(inputs, iters=24, reps=2):
    """Per-execution device time via async dispatch pipelining."""
    import time

    import jax

    x0, x0tp, ebt, wkk, wqp, wv, wo, w1, w2 = _host_prep(inputs)
    if "nc" not in _NC_CACHE:
        _NC_CACHE["nc"] = build_nc()
    nc = _NC_CACHE["nc"]
    in_map = dict(
        x0=np.ascontiguousarray(x0[:B_LOC]),
        x0t=np.ascontiguousarray(x0tp[: B_LOC // 2]),
        ebt=np.ascontiguousarray(ebt[:B_LOC]),
        wkk=wkk, wqp=wqp, wv=wv, wo=wo, w1=w1, w2=w2,
    )
    jfn, in_names, zero_outs = _jit_single_core(nc)
    dev = jax.devices()[0]
    ins_dev = [jax.device_put(np.asarray(in_map[n]), dev) for n in in_names]
    n_zsets = (iters + 2) * reps + 4
    zsets = [
        [jax.device_put(z.copy(), dev) for z in zero_outs] for _ in range(n_zsets)
    ]
    jax.block_until_ready(zsets)
    jax.block_until_ready(ins_dev)
    state = {"zi": 0}

    def run_m(m):
        outs = []
        t0 = time.perf_counter()
        for _ in range(m):
            outs.append(jfn(*ins_dev, *zsets[state["zi"]]))
            state["zi"] += 1
        jax.block_until_ready(outs)
        return time.perf_counter() - t0

    run_m(1)  # warm (compiles)
    t1s, tns = [], []
    for _ in range(reps):
        t1s.append(run_m(1))
        tns.append(run_m(iters))
    marginal_ns = (min(tns) - min(t1s)) / (iters - 1) * 1e9
    return dict(
        est_exec_ns=marginal_ns,
        t1_ns=min(t1s) * 1e9,
        tn_ns=min(tns) * 1e9,
        t1s=t1s,
        tns=tns,
        iters=iters,
    )
